# revision 34
# baseline (speedup 1.0000x reference)
"""Trainium2 Bass kernel for a dense transformer block (B=2, T=2048, C=1024, H=16).

Sequence-sharded with folded causal pairing: core i owns query blocks
{i, 15-i} of each batch. LN1 runs on the host (gains folded into the
projection weights); on-device compute is fp8 DoubleRow everywhere except
LayerNorm statistics and the residual stream.

Attention is fully fp8. K/Q are quantized at eviction (k x16, q x64) and the
score matmuls run DoubleRow with a 64-partition contraction whose upper row
group carries two "kill channel" rows: key blocks are rotated per core on the
host so the causal diagonals land at static slots 0 (window A) and 8
(window B), and out-of-horizon (slot, window) units get -49 added to their
scores pre-exp via the kill channels. This removes all mask multiplies except
one tril tensor-tensor per diagonal. Exp applies a -2.9 shift (cancels in the
folded softmax denominator) so probabilities stay below the device fp8 max of
240 and writes fp8 pT directly; AV runs DoubleRow over key slot pairs.

The MLP runs both matmuls as 3-pass hi+lo fp8 DoubleRow splits
(a@Wa + a@Wb + b@Wa at one PSUM scale) which matches bf16 accuracy at ~40%
of the PE cost. Exp and Gelu activation tables are disjoint, so gelu is kept
out of the exp phases: MLP1 for batch-0 tokens is evicted to bf16 during
attention and gelu'd in one burst afterwards.
"""

import sys

sys.path.insert(0, "/opt/trn_rl_repo")

import ml_dtypes
import numpy as np

import concourse.bacc as bacc
import concourse.tile as tile
from concourse import mybir
from concourse.bass_utils import run_bass_kernel_spmd

F = mybir.dt.float32
BF = mybir.dt.bfloat16
F8 = mybir.dt.float8e4
AF = mybir.ActivationFunctionType
OP = mybir.AluOpType
DR = mybir.MatmulPerfMode.DoubleRow

B, T, C, H, HD = 2, 2048, 1024, 16, 64
BT = B * T
D4 = 4 * C
P = 128
NBLK = T // P            # 16 key blocks of 128 per batch
NCORES = 8
TT = 512                 # token-tile width for the KVQ pass
NTT = T // TT
WS = 64.0                # fp8 weight scale (k/q/p projections)
WSV = 32.0               # fp8 weight scale for V
WK = 16.0                # k storage scale (evict pk * WK/WS)
WQ = 64.0                # q storage scale (evict copy)
ESHIFT = 2.9             # exp shift; cancels in softmax normalization
KILLV = 224.0            # kill magnitude: 224*224/1024 = 49
SCALE_S = 1.0 / (WK * WQ)
WH2 = 16.0               # h2 scale (folded into rstd)
WM = 16.0                # gelu output scale
WW1 = 1024.0             # w1 host scale
WW2 = 2048.0             # w2 host scale

# k tile layout per batch: 8 jt regions of (16 slots x 128 keys), then the
# kill region (16 slots x 128; rows 0/64 = killA, rows 1/65 = killB).
KJT = NBLK * P           # 2048
KKILL = 8 * KJT
KTOT = KKILL + KJT
# qT tile layout per batch: 8 jt regions of 256 q cols, then the kill strip
# (rows 0/64 kill window-A cols 0:128, rows 1/65 kill window-B cols 128:256).
QJT = 2 * P
QSTRIP = 8 * QJT
QTOT = QSTRIP + QJT

_CACHE = {}


def _build_program():
    nc = bacc.Bacc("TRN2", target_bir_lowering=False)

    hT = nc.dram_tensor("hT", [P, 4, 2, BT], F8, kind="ExternalInput")
    hqT = nc.dram_tensor("hqT", [P, 4, 2, 4 * P], F8, kind="ExternalInput")
    wk = nc.dram_tensor("wk", [P, 4, 2, C], F8, kind="ExternalInput")
    wq = nc.dram_tensor("wq", [P, 4, 2, C], F8, kind="ExternalInput")
    wv = nc.dram_tensor("wv", [P, 4, 2, C], F8, kind="ExternalInput")
    wp = nc.dram_tensor("wp", [P, 4, 2, C], F8, kind="ExternalInput")
    w1a = nc.dram_tensor("w1a", [P, 4, 2, D4], F8, kind="ExternalInput")
    w1b = nc.dram_tensor("w1b", [P, 4, 2, D4], F8, kind="ExternalInput")
    w2a = nc.dram_tensor("w2a", [P, 16, 2, C], F8, kind="ExternalInput")
    w2b = nc.dram_tensor("w2b", [P, 16, 2, C], F8, kind="ExternalInput")
    killk = nc.dram_tensor("killk", [P, KJT], F8, kind="ExternalInput")
    killq = nc.dram_tensor("killq", [P, QJT], F8, kind="ExternalInput")
    tril = nc.dram_tensor("tril", [P, P], BF, kind="ExternalInput")
    xq = nc.dram_tensor("xq", [4 * P, C], F, kind="ExternalInput")
    out = nc.dram_tensor("out", [4 * P, C], F, kind="ExternalOutput")

    with tile.TileContext(nc) as tc:
        with tc.tile_pool(name="maskp", bufs=1) as mp_, \
             tc.tile_pool(name="small", bufs=3) as smp, \
             tc.tile_pool(name="pt", bufs=6) as ptp, \
             tc.tile_pool(name="yst", bufs=3) as ystp:
            mm_cm = tc.tile_pool(name="mm", bufs=2, space="PSUM", side="left")
            mmp = mm_cm.__enter__()
            # phase-1-only extra eviction pipeline depth for kvq0
            xtra_cm = tc.tile_pool(name="xtra", bufs=2, space="PSUM",
                                   side="right")
            xtrap = xtra_cm.__enter__()

            wts_cm = tc.tile_pool(name="wts", bufs=1, side="left")
            wtp = wts_cm.__enter__()
            wk_sb = wtp.tile([P, 4, 2, C], F8, tag="wk")
            wv_sb = wtp.tile([P, 4, 2, C], F8, tag="wv")
            wq_sb = wtp.tile([P, 4, 2, C], F8, tag="wq")

            tril_sb = mp_.tile([P, P], BF, tag="tril")
            bneg = mp_.tile([P, 1], F, tag="bneg")
            nc.gpsimd.memset(bneg, -ESHIFT)

            st = {"k": [None] * B, "v": [None] * B, "q": [None] * B,
                  "y": [None] * B}
            st["y"][0] = mp_.tile([P, 4, 2, 2 * P], F8, tag="ysb0",
                                  name="ysb0")
            st["y"][1] = mp_.tile([P, 4, 2, 2 * P], F8, tag="ysb1",
                                  name="ysb1")

            kv0_cm = tc.tile_pool(name="kv0", bufs=1, side="left")
            kv0 = kv0_cm.__enter__()
            st["k"][0] = kv0.tile([P, KTOT], F8, tag="ksb0", name="ksb0")
            st["v"][0] = kv0.tile([P, NBLK, H * 65], F8, tag="vsb0",
                                  name="vsb0")
            st["q"][0] = kv0.tile([P, QTOT], F8, tag="qsb0", name="qsb0")

            kv1_cm = tc.tile_pool(name="kv1", bufs=1, side="right")
            kv1 = kv1_cm.__enter__()
            st["k"][1] = kv1.tile([P, KTOT], F8, tag="ksb1", name="ksb1")
            st["v"][1] = kv1.tile([P, NBLK, H * 65], F8, tag="vsb1",
                                  name="vsb1")
            st["q"][1] = kv1.tile([P, QTOT], F8, tag="qsb1", name="qsb1")

            def load_weights():
                nc.sync.dma_start(out=wk_sb[:, 0:1], in_=wk[:, 0:1, :, :])

            def load_weights2():
                nc.sync.dma_start(out=wk_sb[:, 1:4], in_=wk[:, 1:4, :, :])
                nc.sync.dma_start(out=wv_sb, in_=wv[:, :, :, :])
                for b in range(B):
                    nc.sync.dma_start(
                        out=st["k"][b][:, KKILL:KKILL + KJT],
                        in_=killk[:, :])
                nc.sync.dma_start(out=tril_sb, in_=tril[:, :])
                for b in range(B):
                    nc.sync.dma_start(
                        out=st["q"][b][:, QSTRIP:QSTRIP + QJT],
                        in_=killq[:, :])

            xin0_cm = tc.tile_pool(name="xin0", bufs=2, side="left")
            xp0 = xin0_cm.__enter__()
            # ---- kvq0 (4-deep eviction pipeline via mm+xtra) ----
            _kvq_tiles(nc, 0, st, hT, wk_sb, wv_sb, (mmp, xtrap), xp0,
                       after_first_dma=(load_weights, load_weights2),
                       k_eng=("dve", "act"), v_eng=("dve", "act"))
            nc.sync.dma_start(out=wq_sb, in_=wq[:, :, :, :])
            _q_proj(nc, 0, st, hqT, wq_sb, mmp, xp0)
            xin0_cm.__exit__(None, None, None)
            xtra_cm.__exit__(None, None, None)
            psp_cm = tc.tile_pool(name="ps", bufs=2, space="PSUM", side="right")
            psp = psp_cm.__enter__()
            pyp_cm = tc.tile_pool(name="py", bufs=2, space="PSUM", side="right")
            pyp = pyp_cm.__enter__()

            # ---- interleave 1: kvq1 steps with attn0 heads ----
            xin1_cm = tc.tile_pool(name="xin1", bufs=2, side="left")
            xp1 = xin1_cm.__enter__()
            kvq1_steps = _kvq_step_list(nc, 1, st, hT, hqT, wk_sb, wv_sb,
                                        wq_sb, mmp, xp1)
            A0 = _AttnState(nc, 0, st, tril_sb, bneg, smp, pyp, ptp, ystp,
                            psp)
            A1 = _AttnState(nc, 1, st, tril_sb, bneg, smp, pyp, ptp, ystp,
                            psp, tt_eng=("gpsimd", "vector"))
            for i in range(H):
                kvq1_steps[i]()
                A0.head(i)
            A0.flush()
            for i in range(5):
                A1.head(i)
            xin1_cm.__exit__(None, None, None)
            kv0_cm.__exit__(None, None, None)
            wts_cm.__exit__(None, None, None)

            postsh_cm = tc.tile_pool(name="postsh", bufs=1, side="left")
            postsh = postsh_cm.__enter__()
            eps_sb = postsh.tile([P, 4, C], F, tag="eps")
            h2t_sb = postsh.tile([P, 4, 2, 4 * P], BF, tag="h2t")
            h2a_sb = postsh.tile([P, 4, 2, 4 * P], F8, tag="h2a")
            h2b_sb = postsh.tile([P, 4, 2, 4 * P], F8, tag="h2b")
            pre0_sb = postsh.tile([P, 32, 2 * P], BF, tag="pre0")
            eps128 = postsh.tile([P, 1], F, tag="eps128")
            nc.vector.memset(eps128, 1e-5 / (WH2 * WH2))
            wp_sb = postsh.tile([P, 4, 2, C], F8, tag="wp")
            nc.sync.dma_start(out=wp_sb, in_=wp[:, :, :, :])
            xq_sb = postsh.tile([P, 4, C], F, tag="xqh")
            nc.sync.dma_start(
                out=xq_sb,
                in_=xq.rearrange("(rt p) c -> p rt c", p=P))

            w1s_cm = tc.tile_pool(name="w1stream", bufs=3, side="left")
            w1sp = w1s_cm.__enter__()
            pre_w1 = {}
            for hg in range(2):
                _w1_load(nc, w1sp, w1a, w1b, pre_w1, hg)

            # ---- interleave 2: attn1 heads with wp0 + LN2 + mlp1h0 ----
            tail0 = _wp_chunks(nc, 0, st, xq_sb, eps_sb, wp_sb, mmp, smp)
            tail0 += _ln2_chunks(nc, (0, 1), eps_sb, h2t_sb, h2a_sb, h2b_sb,
                                 eps128, smp)
            tail0 += _mlp1_noact(nc, h2a_sb, h2b_sb, w1a, w1b, pre0_sb,
                                 w1sp, pre_w1, mmp)
            ntail = len(tail0)
            per = (ntail + 10) // 11
            ti = 0
            for i in range(5, H):
                A1.head(i)
                stop = min(ntail, (i - 4) * per)
                while ti < stop:
                    tail0[ti]()
                    ti += 1
            A1.flush()
            while ti < ntail:
                tail0[ti]()
                ti += 1
            kv1_cm.__exit__(None, None, None)

            pyp_cm.__exit__(None, None, None)
            psp_cm.__exit__(None, None, None)

            pacc_cm = tc.tile_pool(name="pacc", bufs=1, space="PSUM",
                                   side="right")
            paccp = pacc_cm.__enter__()
            ptr_cm = tc.tile_pool(name="ptrp", bufs=2, space="PSUM",
                                  side="right")
            ptrp = ptr_cm.__enter__()

            mlpsb_cm = tc.tile_pool(name="mlpsb", bufs=1, side="left")
            mlpp = mlpsb_cm.__enter__()
            aTa_sb = mlpp.tile([P, 16, 2, 4 * P], F8, tag="aTa")
            aTb_sb = mlpp.tile([P, 16, 2, 4 * P], F8, tag="aTb")

            def tail1():
                for fn in _wp_chunks(nc, 1, st, xq_sb, eps_sb, wp_sb, mmp,
                                     smp):
                    fn()
                for fn in _ln2_chunks(nc, (2, 3), eps_sb, h2t_sb, h2a_sb,
                                      h2b_sb, eps128, smp):
                    fn()

            _gelu_burst0(nc, pre0_sb, aTa_sb, aTb_sb, smp, between=tail1)
            _mlp1_half1(nc, h2a_sb, h2b_sb, w1a, w1b, ptrp, w1sp,
                        aTa_sb, aTb_sb, smp)
            mm_cm.__exit__(None, None, None)
            _mlp2_all(nc, tc, eps_sb, w2a, w2b, out, aTa_sb, aTb_sb,
                      paccp, mlpp)
            ptr_cm.__exit__(None, None, None)

            mlpsb_cm.__exit__(None, None, None)
            pacc_cm.__exit__(None, None, None)
            w1s_cm.__exit__(None, None, None)
            postsh_cm.__exit__(None, None, None)

    nc.compile()
    return nc


def _copy(nc, eng, out, in_, scale=None):
    if scale is not None:
        if eng == "dve":
            nc.vector.tensor_scalar_mul(out, in_, scale)
        elif eng == "act":
            nc.scalar.activation(out=out, in_=in_, func=AF.Copy, scale=scale)
        else:
            nc.gpsimd.tensor_scalar_mul(out, in_, scale)
        return
    if eng == "dve":
        nc.vector.tensor_copy(out=out, in_=in_)
    elif eng == "act":
        nc.scalar.activation(out=out, in_=in_, func=AF.Copy)
    else:
        nc.gpsimd.tensor_copy(out=out, in_=in_)


def _kvq_tiles(nc, b, st, hT, wk_sb, wv_sb, pools, xp, after_first_dma=None,
               k_eng=None, v_eng=None):
    v_sb = st["v"][b]
    v_heads = v_sb.rearrange("p r (h w) -> p r h w", w=65)
    for tt in range(NTT):
        ts_ = tt * TT
        ht = xp.tile([P, 4, 2, TT], F8, tag="ht")
        if tt == 0 and after_first_dma is not None:
            after_first_dma[0]()
            nc.sync.dma_start(out=ht[:, 0:1],
                              in_=hT[:, 0:1, :, b * T + ts_:b * T + ts_ + TT])
            nc.sync.dma_start(out=ht[:, 1:4],
                              in_=hT[:, 1:4, :, b * T + ts_:b * T + ts_ + TT])
            after_first_dma[1]()
        else:
            nc.sync.dma_start(out=ht,
                              in_=hT[:, :, :, b * T + ts_:b * T + ts_ + TT])
        if tt == 0:
            nc.gpsimd.memset(v_heads[:, :, :, 64:65], WSV / 16.0)
        for q4 in range(4):
            _kvq_quarter(nc, b, st, ht, wk_sb, wv_sb, pools, tt, q4,
                         k_eng, v_eng)


def _kvq_quarter(nc, b, st, ht, wk_sb, wv_sb, pools, tt, q4, k_eng, v_eng):
    if not isinstance(pools, tuple):
        pools = (pools,)
    k_sb, v_sb = st["k"][b], st["v"][b]
    v_heads = v_sb.rearrange("p r (h w) -> p r h w", w=65)
    ts_ = tt * TT
    for jj in range(2):
        jt = q4 * 2 + jj
        mmp = pools[jj % len(pools)]
        pk = mmp.tile([P, TT], F, tag="mm", name=f"pk{b}_{tt}_{jt}")
        for o in range(4):
            nc.tensor.matmul(pk, wk_sb[:, o, :, jt * P:(jt + 1) * P],
                             ht[:, o], start=(o == 0), stop=(o == 3),
                             perf_mode=DR)
        _copy(nc, k_eng[jt % len(k_eng)],
              k_sb[:, jt * KJT + ts_:jt * KJT + ts_ + TT], pk,
              scale=WK / WS)
    t4 = q4
    ridx = tt * (TT // P) + t4
    for nh in range(2):
        mmp = pools[nh % len(pools)]
        pv = mmp.tile([P, 512], F, tag="mm", name=f"pv{b}_{ridx}_{nh}")
        for o in range(4):
            nc.tensor.matmul(pv, ht[:, o, :, t4 * P:(t4 + 1) * P],
                             wv_sb[:, o, :, nh * 512:(nh + 1) * 512],
                             start=(o == 0), stop=(o == 3),
                             perf_mode=DR)
        pvv = pv.rearrange("p (h d) -> p h d", d=HD)
        _copy(nc, v_eng[nh % len(v_eng)],
              v_heads[:, ridx, nh * 8:(nh + 1) * 8, 0:HD], pvv)


def _q_proj(nc, b, st, hqT, wq_sb, mmp, xp):
    qT_sb = st["q"][b]
    hq = xp.tile([P, 4, 2, 2 * P], F8, tag="hq")
    nc.sync.dma_start(out=hq, in_=hqT[:, :, :, b * 2 * P:(b + 1) * 2 * P])
    for jt in range(8):
        pq = mmp.tile([P, 2 * P], F, tag="mm", name=f"pq{b}_{jt}")
        for o in range(4):
            nc.tensor.matmul(pq, wq_sb[:, o, :, jt * P:(jt + 1) * P],
                             hq[:, o], start=(o == 0), stop=(o == 3),
                             perf_mode=DR)
        _copy(nc, "dve", qT_sb[:, jt * QJT:(jt + 1) * QJT], pq)


def _kvq_step_list(nc, b, st, hT, hqT, wk_sb, wv_sb, wq_sb, mmp, xp):
    """kvq for batch b as 16 callables; evictions stay off the Act engine
    (it runs the other batch's exp concurrently). Only DVE and Act can
    read PSUM, so everything lands on DVE here."""
    k_eng = ("dve",)
    v_eng = ("dve",)
    hts = {}
    steps = []
    for tt in range(NTT):
        for q4 in range(4):
            def mk(tt=tt, q4=q4):
                def fn():
                    if q4 == 0:
                        ts_ = tt * TT
                        ht = xp.tile([P, 4, 2, TT], F8, tag="ht")
                        nc.sync.dma_start(
                            out=ht,
                            in_=hT[:, :, :, b * T + ts_:b * T + ts_ + TT])
                        if tt == 0:
                            v_heads = st["v"][b].rearrange(
                                "p r (h w) -> p r h w", w=65)
                            nc.gpsimd.memset(v_heads[:, :, :, 64:65],
                                             WSV / 16.0)
                        hts[tt] = ht
                    _kvq_quarter(nc, b, st, hts[tt], wk_sb, wv_sb, mmp, tt,
                                 q4, k_eng, v_eng)
                    if tt == NTT - 1 and q4 == 3:
                        _q_proj(nc, b, st, hqT, wq_sb, mmp, xp)
                return fn
            steps.append(mk())
    return steps


class _AttnState:
    """Software-pipelined fp8 DoubleRow attention for one batch."""

    def __init__(self, nc, b, st, tril_sb, bneg, smp, pyp, ptp, ystp, psp,
                 tt_eng=("vector", "gpsimd")):
        self.nc = nc
        self.b = b
        self.st = st
        self.tril_sb = tril_sb
        self.bneg = bneg
        self.smp, self.pyp, self.ptp = smp, pyp, ptp
        self.ystp, self.psp = ystp, psp
        self.tt_eng = tt_eng
        self.pending = []

    def head(self, h):
        pts = self._scores(h)
        self.pending.append((h, pts))
        if len(self.pending) > 2:
            self._av(*self.pending.pop(0))

    def flush(self):
        while self.pending:
            self._av(*self.pending.pop(0))

    def _scores(self, h):
        nc, b = self.nc, self.b
        k_sb, qT_sb = self.st["k"][b], self.st["q"][b]
        po = (h % 2) * 64
        jt = h // 2
        kdel = KKILL - jt * KJT          # slot region -> kill region
        qdel = QSTRIP - jt * QJT         # q region -> kill strip

        def k_ap(slot):
            base = jt * KJT + slot * P
            nr = kdel // P
            v = k_sb[:, base:base + kdel + P].rearrange(
                "p (r k) -> p r k", r=nr + 1)
            return v[po:po + 64, 0:nr + 1:nr, :]

        def q_ap(off, width):
            base = jt * QJT + off
            nr = qdel // width
            v = qT_sb[:, base:base + qdel + width].rearrange(
                "p (r q) -> p r q", r=nr + 1)
            return v[po:po + 64, 0:nr + 1:nr, :]

        pts = []
        # slots 0..7 x both windows (256q) in two 2-bank tiles
        for m2 in range(2):
            ps_ = self.psp.tile([P, 4, 256], F, tag="ps",
                                name=f"ps{b}_{h}_{m2}")
            for j in range(4):
                nc.tensor.matmul(ps_[:, j, :], k_ap(m2 * 4 + j),
                                 q_ap(0, 2 * P),
                                 start=True, stop=True, perf_mode=DR)
            pT = self.ptp.tile([P, 4, 256], F8, tag="pT")
            nc.scalar.activation(out=pT, in_=ps_, func=AF.Exp,
                                 scale=SCALE_S, bias=self.bneg)
            if m2 == 0:
                eng = nc.vector if self.tt_eng[0] == "vector" else nc.gpsimd
                eng.tensor_tensor(out=pT[:, 0, 0:P], in0=pT[:, 0, 0:P],
                                  in1=self.tril_sb, op=OP.mult)
            pts.append(pT)
        # slots 8..15 x window B (128q) in one 2-bank tile
        ps_ = self.psp.tile([P, 8, P], F, tag="ps", name=f"psB{b}_{h}")
        for j in range(8):
            nc.tensor.matmul(ps_[:, j, :], k_ap(8 + j), q_ap(P, P),
                             start=True, stop=True, perf_mode=DR)
        pT = self.ptp.tile([P, 8, P], F8, tag="pTB")
        nc.scalar.activation(out=pT, in_=ps_, func=AF.Exp,
                             scale=SCALE_S, bias=self.bneg)
        eng = nc.vector if self.tt_eng[1] == "vector" else nc.gpsimd
        eng.tensor_tensor(out=pT[:, 0, :], in0=pT[:, 0, :],
                          in1=self.tril_sb, op=OP.mult)
        pts.append(pT)
        return pts

    def _av(self, h, pts):
        nc, b = self.nc, self.b
        v_sb, yT_sb = self.st["v"][b], self.st["y"][b]
        py = self.pyp.tile([65, 2 * P], F, tag="py")
        for m2 in range(2):
            for j in range(2):
                sp = m2 * 4 + j * 2
                nc.tensor.matmul(
                    py, v_sb[:, sp:sp + 2, h * 65:h * 65 + 65],
                    pts[m2][:, 2 * j:2 * j + 2, :],
                    start=(sp == 0), stop=False, perf_mode=DR)
        for j in range(4):
            nc.tensor.matmul(
                py[:, P:2 * P],
                v_sb[:, 8 + 2 * j:10 + 2 * j, h * 65:h * 65 + 65],
                pts[2][:, 2 * j:2 * j + 2, :],
                start=False, stop=(j == 3), perf_mode=DR)
        rec = self.smp.tile([1, 2 * P], F, tag="rec")
        nc.vector.reciprocal(out=rec, in_=py[64:65, :])
        recb = self.smp.tile([64, 2 * P], F, tag="recb")
        nc.gpsimd.partition_broadcast(recb, rec)
        yev = self.ystp.tile([64, 2 * P], F8, tag="yev")
        nc.vector.tensor_tensor(out=yev, in0=py[0:64, :], in1=recb,
                                op=OP.mult)
        nc.sync.dma_start(
            out=yT_sb[(h % 2) * 64:(h % 2) * 64 + 64, h // 4, (h // 2) % 2, :],
            in_=yev)


def _wp_chunks(nc, b, st, xq_sb, eps_sb, wp_sb, mmp, smp):
    yT_sb = st["y"][b]
    chunks = []
    for th in range(2):
        for nh in range(2):
            def mk(th=th, nh=nh):
                def fn():
                    rt = 2 * b + th
                    pr = mmp.tile([P, 512], F, tag="mm",
                                  name=f"pr{b}_{th}_{nh}")
                    for o in range(4):
                        nc.tensor.matmul(
                            pr, yT_sb[:, o, :, th * P:(th + 1) * P],
                            wp_sb[:, o, :, nh * 512:(nh + 1) * 512],
                            start=(o == 0), stop=(o == 3), perf_mode=DR)
                    nc.vector.scalar_tensor_tensor(
                        out=eps_sb[:, rt, nh * 512:(nh + 1) * 512],
                        in0=pr, scalar=1.0 / (WS * 16.0),
                        in1=xq_sb[:, rt, nh * 512:(nh + 1) * 512],
                        op0=OP.mult, op1=OP.add)
                return fn
            chunks.append(mk())
    return chunks


def _ln2_chunks(nc, rts, eps_sb, h2t_sb, h2a_sb, h2b_sb, eps128, h2p):
    """LN2 (emitting WH2*h2 in bf16), DMA transpose, fp8 hi/lo split.
    The two rts' Act sqrts are emitted adjacently (one table excursion)."""
    mvs = {}

    def mk_stats(rt):
        def fn():
            stats = h2p.tile([P, 2, 6], F, tag="st2")
            nc.vector.bn_stats(out=stats[:, 0, :], in_=eps_sb[:, rt, 0:512])
            nc.vector.bn_stats(out=stats[:, 1, :],
                               in_=eps_sb[:, rt, 512:1024])
            mv = h2p.tile([P, 2], F, tag=f"mv2_{rt % 2}")
            nc.vector.bn_aggr(out=mv, in_=stats)
            mvs[rt] = mv
        return fn

    def mk_sqrts():
        def fn():
            for rt in rts:
                sd = h2p.tile([P, 1], F, tag=f"sd2_{rt % 2}")
                nc.scalar.activation(out=sd, in_=mvs[rt][:, 1:2],
                                     func=AF.Sqrt,
                                     scale=1.0 / (WH2 * WH2), bias=eps128)
                mvs[(rt, "sd")] = sd
        return fn

    def mk_norm(rt):
        def fn():
            rstd2 = h2p.tile([P, 1], F, tag=f"rstd2_{rt % 2}")
            nc.vector.reciprocal(out=rstd2, in_=mvs[(rt, "sd")])
            h2 = h2p.tile([P, C], BF, tag=f"h2_{rt % 2}")
            nc.vector.tensor_scalar(out=h2, in0=eps_sb[:, rt, :],
                                    scalar1=mvs[rt][:, 0:1], scalar2=rstd2,
                                    op0=OP.subtract, op1=OP.mult)
            nc.sync.dma_start_transpose(
                out=h2t_sb.rearrange("p o r t -> p (o r) t")[
                    :, :, rt * P:(rt + 1) * P], in_=h2)
        return fn

    def mk_split(rt):
        def fn():
            s3 = h2t_sb[:, :, :, rt * P:(rt + 1) * P]
            a3 = h2a_sb[:, :, :, rt * P:(rt + 1) * P]
            b3 = h2b_sb[:, :, :, rt * P:(rt + 1) * P]
            nc.gpsimd.tensor_copy(out=a3, in_=s3)
            nc.vector.tensor_tensor(out=b3, in0=s3, in1=a3, op=OP.subtract)
        return fn

    chunks = [mk_stats(rt) for rt in rts]
    chunks.append(mk_sqrts())
    for rt in rts:
        chunks.append(mk_norm(rt))
        chunks.append(mk_split(rt))
    return chunks


def _w1_load(nc, w1sp, w1a, w1b, pre_w1, hg):
    if hg in pre_w1:
        return
    w1c = w1sp.tile([P, 4, 2, 512], F8, tag="w1c")
    nc.sync.dma_start(out=w1c, in_=w1a[:, :, :, hg * 512:(hg + 1) * 512])
    w1d = w1sp.tile([P, 4, 2, 512], F8, tag="w1d")
    nc.sync.dma_start(out=w1d, in_=w1b[:, :, :, hg * 512:(hg + 1) * 512])
    pre_w1[hg] = (w1c, w1d)


def _mlp1_group(nc, h2a_sb, h2b_sb, w1pair, og, cs, width, pa):
    """12 DR matmuls: gelu-input chunks ht=(2og, 2og+1) for token cols
    cs:cs+width into psum pa [128, 2, width] (3-pass hi/lo split)."""
    w1c, w1d = w1pair
    for half in range(2):
        ht = og * 2 + half
        col = (ht % 4) * P
        first = True
        for pi, (xa, xw) in enumerate(
                ((h2a_sb, w1c), (h2a_sb, w1d), (h2b_sb, w1c))):
            for o in range(4):
                nc.tensor.matmul(
                    pa[:, half, :], xw[:, o, :, col:col + P],
                    xa[:, o, :, cs:cs + width],
                    start=first, stop=(pi == 2 and o == 3),
                    perf_mode=DR)
                first = False


def _mlp1_noact(nc, h2a_sb, h2b_sb, w1a, w1b, pre0_sb, w1sp, pre_w1, mmp):
    """MLP1 matmuls for token half 0, evicted to bf16 pre-activations."""
    steps = []
    for hg in range(8):
        def mk_load(hg=hg):
            def fn():
                _w1_load(nc, w1sp, w1a, w1b, pre_w1, hg)
                _w1_load(nc, w1sp, w1a, w1b, pre_w1, min(hg + 2, 7))
            return fn
        steps.append(mk_load())
        for og2 in range(2):
            def mk(hg=hg, og2=og2):
                def fn():
                    og = hg * 2 + og2
                    pa = mmp.tile([P, 512], F, tag="mm",
                                  name=f"pam{og}").rearrange(
                        "p (r t) -> p r t", r=2)
                    _mlp1_group(nc, h2a_sb, h2b_sb, pre_w1[hg], og,
                                0, 2 * P, pa)
                    _copy(nc, "dve", pre0_sb[:, og * 2:og * 2 + 2, :], pa)
                return fn
            steps.append(mk())
    return steps


def _gelu_burst0(nc, pre0_sb, aTa_sb, aTb_sb, smp, between=None):
    """Gelu token half 0 from SBUF in wide tiles, then hi/lo split."""
    aTa = aTa_sb.rearrange("p o r t -> p (o r) t")
    aTb = aTb_sb.rearrange("p o r t -> p (o r) t")
    for g in range(8):
        tmp = smp.tile([P, 4, 2 * P], BF, tag=f"gtmp{g % 2}")
        nc.scalar.activation(
            out=tmp, in_=pre0_sb[:, 4 * g:4 * g + 4, :],
            func=AF.Gelu, scale=1.0 / (WH2 * WW1))
        for i in range(4):
            ht = 4 * g + i
            nc.gpsimd.tensor_scalar_mul(aTa[:, ht, 0:2 * P],
                                        tmp[:, i, :], WM)
            nc.vector.scalar_tensor_tensor(
                out=aTb[:, ht, 0:2 * P], in0=tmp[:, i, :], scalar=WM,
                in1=aTa[:, ht, 0:2 * P], op0=OP.mult, op1=OP.subtract)
        if between is not None and g == 0:
            between()
            between = None


def _mlp1_half1(nc, h2a_sb, h2b_sb, w1a, w1b, ptrp, w1sp, aTa_sb, aTb_sb,
                smp):
    """MLP1 half 1 with direct gelu eviction + hi/lo split."""
    aTa = aTa_sb.rearrange("p o r t -> p (o r) t")
    aTb = aTb_sb.rearrange("p o r t -> p (o r) t")
    pre_w1 = {}
    _w1_load(nc, w1sp, w1a, w1b, pre_w1, 0)
    _w1_load(nc, w1sp, w1a, w1b, pre_w1, 1)
    for hg in range(8):
        if hg + 2 < 8:
            _w1_load(nc, w1sp, w1a, w1b, pre_w1, hg + 2)
        for og2 in range(2):
            og = hg * 2 + og2
            pa = ptrp.tile([P, 512], F, tag="pa",
                           name=f"pa1_{og}").rearrange(
                "p (r t) -> p r t", r=2)
            _mlp1_group(nc, h2a_sb, h2b_sb, pre_w1[hg], og,
                        2 * P, 2 * P, pa)
            tmp = smp.tile([P, 2, 2 * P], BF, tag=f"g1tmp{og % 2}")
            nc.scalar.activation(out=tmp, in_=pa, func=AF.Gelu,
                                 scale=1.0 / (WH2 * WW1))
            for half in range(2):
                ht = og * 2 + half
                nc.gpsimd.tensor_scalar_mul(
                    aTa[:, ht, 2 * P:4 * P], tmp[:, half, :], WM)
                nc.vector.scalar_tensor_tensor(
                    out=aTb[:, ht, 2 * P:4 * P], in0=tmp[:, half, :],
                    scalar=WM, in1=aTa[:, ht, 2 * P:4 * P],
                    op0=OP.mult, op1=OP.subtract)


def _mlp2_all(nc, tc, eps_sb, w2a, w2b, out, aTa_sb, aTb_sb, accp, mlpp):
    """MLP2 for all 4 row-tiles and both column halves at once (8 PSUM
    banks): single w2 stream, both nh interleaved per o chunk so only the
    last aT chunk gates the tail."""
    out_sb = mlpp.tile([P, 4, C], F, tag="outsb")
    with tc.tile_pool(name="w2stream", bufs=4, side="left") as wsp2:
        for nh in range(2):
            pms = [accp.tile([P, 512], F, tag=f"acc{rt}",
                             name=f"pm{nh}_{rt}")
                   for rt in range(4)]
            for o in range(16):
                w2c = wsp2.tile([P, 2, 512], F8, tag="w2c",
                                name=f"w2c{nh}_{o}")
                nc.sync.dma_start(
                    out=w2c, in_=w2a[:, o, :, nh * 512:(nh + 1) * 512])
                w2d = wsp2.tile([P, 2, 512], F8, tag="w2d",
                                name=f"w2d{nh}_{o}")
                nc.sync.dma_start(
                    out=w2d, in_=w2b[:, o, :, nh * 512:(nh + 1) * 512])
                for rt in range(4):
                    cs = rt * P
                    for pi, (xa, xw) in enumerate(
                            ((aTa_sb, w2c), (aTa_sb, w2d), (aTb_sb, w2c))):
                        nc.tensor.matmul(
                            pms[rt], xa[:, o, :, cs:cs + P], xw,
                            start=(o == 0 and pi == 0),
                            stop=(o == 15 and pi == 2),
                            perf_mode=DR)
            for rt in range(4):
                nc.vector.scalar_tensor_tensor(
                    out=out_sb[:, rt, nh * 512:(nh + 1) * 512],
                    in0=pms[rt], scalar=1.0 / (WM * WW2),
                    in1=eps_sb[:, rt, nh * 512:(nh + 1) * 512],
                    op0=OP.mult, op1=OP.add)
                nc.sync.dma_start(
                    out=out.rearrange("(rt p) c -> p rt c", p=P)[
                        :, rt:rt + 1, nh * 512:(nh + 1) * 512],
                    in_=out_sb[:, rt:rt + 1, nh * 512:(nh + 1) * 512])


def _dr_pack(m, og=4):
    """[C_in, N] -> [128, og, 2, N]: c -> (c//256, (c%256)//128, c%128)."""
    cin, n = m.shape
    assert cin == og * 256
    return np.ascontiguousarray(
        m.reshape(og, 2, P, n).transpose(2, 0, 1, 3))


def _host_prep(inputs):
    ii = {k: np.asarray(v, dtype=np.float32) for k, v in inputs.items()}
    x = ii["x"]
    for bias in ("bq", "bk", "bv", "bp", "b1", "b2", "ln1_b", "ln2_b"):
        assert np.allclose(ii[bias], 0.0), f"nonzero {bias} unsupported"

    e4 = ml_dtypes.float8_e4m3fn
    xflat = x.reshape(BT, C)
    mu = xflat.mean(axis=1, keepdims=True)
    var = ((xflat - mu) ** 2).mean(axis=1, keepdims=True)
    h = (xflat - mu) / np.sqrt(var + 1e-5)

    g1 = ii["ln1_g"][:, None]
    wq_f = (g1 * ii["Wq"] / np.sqrt(HD)).astype(np.float32)
    wk_f = (g1 * ii["Wk"]).astype(np.float32)
    wv_f = (g1 * ii["Wv"]).astype(np.float32)
    g2 = ii["ln2_g"][:, None]
    w1_f = (g2 * ii["W1"]).astype(np.float32)

    w1s = w1_f * WW1
    w1a = w1s.astype(e4)
    w1b = (w1s - w1a.astype(np.float32)).astype(e4)
    w2s = ii["W2"] * WW2
    w2a = w2s.astype(e4)
    w2b = (w2s - w2a.astype(np.float32)).astype(e4)

    kk = np.arange(P)[:, None]
    jj = np.arange(P)[None, :]
    tril = np.ascontiguousarray((kk <= jj).astype(ml_dtypes.bfloat16))

    shared = {
        "wk": _dr_pack(wk_f * WS).astype(e4),
        "wq": _dr_pack(wq_f * WS).astype(e4),
        "wv": _dr_pack(wv_f * WSV).astype(e4),
        "wp": _dr_pack(ii["Wp"] * WS).astype(e4),
        "w1a": _dr_pack(w1a.astype(np.float32)).astype(e4),
        "w1b": _dr_pack(w1b.astype(np.float32)).astype(e4),
        "w2a": _dr_pack(w2a.astype(np.float32), og=16).astype(e4),
        "w2b": _dr_pack(w2b.astype(np.float32), og=16).astype(e4),
        "tril": tril,
    }

    in_maps = []
    core_rows = []
    for core in range(NCORES):
        i = core
        qbA, qbB = i, NBLK - 1 - i
        # slot rotation: slot 0 = A diag; 1..i = blocks 0..i-1; slot 8 =
        # B diag; remaining blocks fill the rest in order.
        slots = [qbA] + list(range(qbA))
        rest = [kb for kb in range(NBLK) if kb not in slots and kb != qbB]
        slots = slots + rest[:7 - i] + [qbB] + rest[7 - i:]
        assert len(slots) == NBLK and sorted(slots) == list(range(NBLK))
        assert slots[0] == qbA and slots[8] == qbB

        killA = np.zeros(NBLK, np.float32)
        killB = np.zeros(NBLK, np.float32)
        for s, kb in enumerate(slots):
            if s < 8 and s != 0 and kb > qbA:
                killA[s] = 1.0
            if kb > qbB:
                killB[s] = 1.0

        killk = np.zeros((P, NBLK, P), np.float32)
        for r0 in (0, 64):
            killk[r0, :, :] = KILLV * killA[:, None]
            killk[r0 + 1, :, :] = KILLV * killB[:, None]
        killq = np.zeros((P, 2, P), np.float32)
        for r0 in (0, 64):
            killq[r0, 0, :] = -KILLV
            killq[r0 + 1, 1, :] = -KILLV

        perm = np.concatenate(
            [b * T + np.concatenate(
                [kb * P + np.arange(P) for kb in slots])
             for b in range(B)])
        hT_core = np.ascontiguousarray(h[perm].T)

        rows = np.concatenate([
            b * T + qb * P + np.arange(P)
            for b in range(B) for qb in (qbA, qbB)])
        core_rows.append(rows)
        xq_i = np.ascontiguousarray(xflat[rows])
        hq_i = np.ascontiguousarray(h[rows].T)

        in_maps.append(dict(
            shared, xq=xq_i,
            hT=_dr_pack(hT_core).astype(e4),
            hqT=_dr_pack(hq_i).astype(e4),
            killk=np.ascontiguousarray(
                killk.reshape(P, NBLK * P).astype(e4)),
            killq=np.ascontiguousarray(
                killq.reshape(P, QJT).astype(e4))))
    return in_maps, core_rows


def kernel(**inputs):
    if "nc" not in _CACHE:
        _CACHE["nc"] = _build_program()
    nc = _CACHE["nc"]
    in_maps, core_rows = _host_prep(inputs)
    res = run_bass_kernel_spmd(nc, in_maps, core_ids=list(range(NCORES)))
    out = np.empty((BT, C), np.float32)
    for core in range(NCORES):
        out[core_rows[core]] = res.results[core]["out"]
    return out.reshape(B, T, C)


if __name__ == "__main__":
    print("module loads OK")
